# revision 5
# baseline (speedup 1.0000x reference)
"""GraphSAGE 2-layer GNN kernel for Trainium2 (8 NeuronCores, SPMD).

Strategy (dst-sharded graph parallel, v2):
  - Nodes are permuted (degree-balanced round-robin) into 392 tiles of 128
    nodes; each core owns 49 tiles.  Tables (x, h) live in DRAM in a
    "chunk-major" row space: 4 chunks of tiles (13/13/13/10 per core), each
    chunk holding all 8 cores' tiles contiguously, so the inter-layer
    AllGather runs as 4 pipelined chunk collectives that overlap layer-1
    compute.
  - int16 gather indices can't span all 50176 rows, so gathers use two
    windows aligned to chunk boundaries: A = rows [0, 26624) (chunks 0-1),
    B = rows [26624, 50176) (chunks 2-3).
  - Per (tile, window) one dma_gather call pulls that tile's source rows
    (bf16, 256 B/row) from the replicated table; variable per-tile chunk
    counts (max across cores, so all 8 cores share one program) keep
    padding low.  A larger SWDGE descriptor ring (dynamic_dma_scratch_size)
    allows ~1700-index calls.
  - Scatter-into-dst is one-hot matmul: per tile ONE DVE is_equal builds the
    combined [128, C_t, 128] one-hot (edge-slot x dst-slot, scaled later by
    1/deg), and C_t chunk matmuls accumulate aggT[d,:] in PSUM.
  - Layer output stays transposed (hT = W_self.T @ xT + W_neigh.T @ aggmeanT
    + b) in bf16; node-major bf16 copies for the collective come from a
    per-tile SBUF DMA transpose.

The final output is returned transposed per core ([64, 6272] fp32); the
host assembles/unpermutes to the full [50000, 64] result.
"""

import numpy as np
import ml_dtypes

N = 50000
E = 800000
D_IN = 128
D_HID = 128
D_OUT = 64
CORES = 8
P = 128

NT = 49                    # node tiles per core
NPC = NT * P               # padded nodes per core (6272)
NTILES = CORES * NT        # 392 total tiles
NPAD = CORES * NPC         # 50176 padded rows total

CH_T0 = [0, 13, 26, 39, 49]          # per-core tile chunk boundaries
NCH = 4
CH_BASE = [0, 13312, 26624, 39936]   # table row base of each chunk
A_END = 26624                        # window A rows [0, A_END)
B_OFF = 26624                        # window B rows [B_OFF, NPAD)

SCRATCH = 16384            # SWDGE descriptor carveout bytes/partition
MAXG = 1024                # max indices per dma_gather call

BF16 = ml_dtypes.bfloat16


def _prep(x, src, dst, W1_self, W1_neigh, b1, W2_self, W2_neigh, b2):
    """Host-side preprocessing: permutation, edge slotting, per-core arrays."""
    x = np.asarray(x, np.float32)
    src = np.asarray(src, np.int64)
    dst = np.asarray(dst, np.int64)

    deg = np.bincount(dst, minlength=N)
    invdeg = 1.0 / np.maximum(deg, 1).astype(np.float32)

    # Degree-balanced node -> (tile, slot): round-robin nodes in descending-
    # degree order over all 392 tiles. g = tile-major padded id.
    order = np.argsort(-deg, kind="stable")
    ranks = np.arange(N)
    g_of_node = np.empty(N, np.int64)
    g_of_node[order] = (ranks % NTILES) * P + ranks // NTILES
    node_of_g = np.full(NPAD, -1, np.int64)
    node_of_g[g_of_node] = np.arange(N)

    # chunk-major physical table row for every g
    g_all = np.arange(NPAD)
    core_g = g_all // NPC
    pos_g = (g_all % NPC) // P
    slot_g = g_all % P
    ch_of_pos = np.searchsorted(CH_T0, pos_g, side="right") - 1
    ch_sz = np.diff(CH_T0)
    row_of_g = (np.asarray(CH_BASE)[ch_of_pos]
                + core_g * ch_sz[ch_of_pos] * P
                + (pos_g - np.asarray(CH_T0)[ch_of_pos]) * P + slot_g)

    gsrc = g_of_node[src]
    gdst = g_of_node[dst]
    rsrc = row_of_g[gsrc]

    core_e = gdst // NPC
    t_e = (gdst % NPC) // P
    dloc_e = gdst % P
    half_e = (rsrc >= A_END).astype(np.int64)

    # per (core, tile, half) counts -> shared per-tile chunk counts
    key = ((core_e * NT + t_e) * 2 + half_e).astype(np.int64)
    counts = np.bincount(key, minlength=NTILES * 2).reshape(CORES, NT, 2)
    CA = np.maximum(1, np.ceil(counts[:, :, 0].max(axis=0) / P).astype(int))
    CB = np.maximum(1, np.ceil(counts[:, :, 1].max(axis=0) / P).astype(int))
    CT = CA + CB
    off = np.zeros(NT + 1, np.int64)
    off[1:] = np.cumsum(CT) * P           # slot offset of each tile region
    TS = int(off[-1])                      # slots per core
    CMAX = int(CT.max())

    # slot assignment: stable sort edges by key, place within group
    edge_order = np.argsort(key, kind="stable")
    key_s = key[edge_order]
    starts = np.zeros(NTILES * 2, np.int64)
    flat_counts = counts.reshape(-1)
    starts[1:] = np.cumsum(flat_counts)[:-1]
    within = np.arange(E) - starts[key_s]

    t_s = (key_s // 2) % NT
    half_s = key_s % 2
    slot_s = np.where(half_s == 0,
                      off[t_s] + within,
                      off[t_s] + CA[t_s] * P + within)
    core_s = key_s // (2 * NT)

    idx_arr = np.zeros((CORES, TS), np.int16)
    # pad slots: dloc sentinel 300 never matches any column 0..127 (exact
    # in bf16), so padded gather rows contribute nothing to the one-hot MM
    dloc_arr = np.full((CORES, TS), 300.0, np.float32)
    flat = core_s * TS + slot_s
    lidx_e = rsrc - half_e * B_OFF
    assert lidx_e.max() < 32768 and lidx_e.min() >= 0
    idx_arr.reshape(-1)[flat] = lidx_e[edge_order].astype(np.int16)
    dloc_arr.reshape(-1)[flat] = dloc_e[edge_order].astype(np.float32)

    # wrapped index layout: slot i -> [i % 16, i // 16], tiled to 128 rows
    idx_w = np.ascontiguousarray(
        np.tile(idx_arr.reshape(CORES, TS // 16, 16).transpose(0, 2, 1), (1, 8, 1))
    )
    # per-chunk scalar layout: slot i -> [i % 128, i // 128]
    dloc_w = np.ascontiguousarray(
        dloc_arr.reshape(CORES, TS // P, P).transpose(0, 2, 1)).astype(BF16)
    # per-node 1/deg, broadcast across partitions, per core shard [128, NPC]
    invdeg_pad = np.zeros(NPAD, np.float32)
    invdeg_pad[g_of_node] = invdeg
    invdb = [np.ascontiguousarray(
        np.tile(invdeg_pad[c * NPC:(c + 1) * NPC], (P, 1)))
        for c in range(CORES)]

    # feature tables / shards
    xpad = np.zeros((NPAD, D_IN), np.float32)
    xpad[g_of_node] = x
    x_tab = np.zeros((NPAD, D_IN), np.float32)
    x_tab[row_of_g] = xpad
    x_bf = x_tab.astype(BF16)
    xT_shards = [np.ascontiguousarray(xpad[c * NPC:(c + 1) * NPC].T).astype(BF16)
                 for c in range(CORES)]

    iota = np.tile(np.arange(P, dtype=np.float32), (P, CMAX)).astype(BF16)

    meta = dict(CA=tuple(int(v) for v in CA), CB=tuple(int(v) for v in CB),
                off=tuple(int(v) for v in off), TS=TS, CMAX=CMAX,
                node_of_g=node_of_g)

    common = {
        "x_bf": x_bf,
        "iota": np.ascontiguousarray(iota),
        "W1s": np.ascontiguousarray(np.asarray(W1_self, np.float32).astype(BF16)),
        "W1n": np.ascontiguousarray(np.asarray(W1_neigh, np.float32).astype(BF16)),
        "b1": np.ascontiguousarray(np.asarray(b1, np.float32).reshape(D_HID, 1)),
        "W2s": np.ascontiguousarray(np.asarray(W2_self, np.float32).astype(BF16)),
        "W2n": np.ascontiguousarray(np.asarray(W2_neigh, np.float32).astype(BF16)),
        "b2": np.ascontiguousarray(np.asarray(b2, np.float32).reshape(D_OUT, 1)),
    }
    per_core = []
    for c in range(CORES):
        m = dict(common)
        m["xT"] = xT_shards[c]
        m["idx_w"] = idx_w[c]
        m["dloc"] = dloc_w[c]
        m["invdb"] = invdb[c]
        per_core.append(m)
    return per_core, meta


def _build(meta):
    """Build the SPMD Bass program (same NEFF for all 8 cores)."""
    import concourse.bacc as bacc
    import concourse.mybir as mybir
    import concourse.tile as tile

    CA, CB, off = meta["CA"], meta["CB"], meta["off"]
    TS, CMAX = meta["TS"], meta["CMAX"]
    f32 = mybir.dt.float32
    bf16 = mybir.dt.bfloat16
    i16 = mybir.dt.int16
    AF = mybir.ActivationFunctionType
    ALU = mybir.AluOpType

    nc = bacc.Bacc(None, target_bir_lowering=False, debug=False,
                   num_devices=CORES, num_swdge_queues=4,
                   dynamic_dma_scratch_size=SCRATCH)

    # I/O
    x_bf_t = nc.dram_tensor("x_bf", [NPAD, D_IN], bf16, kind="ExternalInput")
    xT_t = nc.dram_tensor("xT", [P, NPC], bf16, kind="ExternalInput")
    idx_t = nc.dram_tensor("idx_w", [P, TS // 16], i16, kind="ExternalInput")
    dloc_t = nc.dram_tensor("dloc", [P, TS // P], bf16, kind="ExternalInput")
    invdb_t = nc.dram_tensor("invdb", [P, NPC], f32, kind="ExternalInput")
    iota_t = nc.dram_tensor("iota", [P, CMAX * P], bf16, kind="ExternalInput")
    W1s_t = nc.dram_tensor("W1s", [D_IN, D_HID], bf16, kind="ExternalInput")
    W1n_t = nc.dram_tensor("W1n", [D_IN, D_HID], bf16, kind="ExternalInput")
    b1_t = nc.dram_tensor("b1", [D_HID, 1], f32, kind="ExternalInput")
    W2s_t = nc.dram_tensor("W2s", [D_HID, D_OUT], bf16, kind="ExternalInput")
    W2n_t = nc.dram_tensor("W2n", [D_HID, D_OUT], bf16, kind="ExternalInput")
    b2_t = nc.dram_tensor("b2", [D_OUT, 1], f32, kind="ExternalInput")
    out_t = nc.dram_tensor("outT", [D_OUT, NPC], f32, kind="ExternalOutput")

    h_shard_t = nc.dram_tensor("h_shard", [NPC, D_HID], bf16)
    h_table_t = nc.dram_tensor("h_table", [NPAD, D_HID], bf16,
                               addr_space="Shared")

    CAMAX = max(CA)
    CBMAX = max(CB)

    with tile.TileContext(nc) as tc:
        with (
            tc.tile_pool(name="const", bufs=1) as cpool,
            tc.tile_pool(name="msgsA", bufs=6) as poolA,
            tc.tile_pool(name="msgsB", bufs=6) as poolB,
            tc.tile_pool(name="oh", bufs=4) as pool_oh,
            tc.tile_pool(name="aggm", bufs=4) as pool_aggm,
            tc.tile_pool(name="small", bufs=3) as pool_small,
            tc.tile_pool(name="psA", bufs=3, space="PSUM") as psumA,
            tc.tile_pool(name="psH", bufs=2, space="PSUM") as psumH,
        ):
            # ---- persistent SBUF state -------------------------------------
            iota_sb = cpool.tile([P, CMAX, P], bf16)
            nc.sync.dma_start(iota_sb[:, :, :], iota_t[:].rearrange(
                "p (c n) -> p c n", n=P))
            idx_sb = cpool.tile([P, TS // 16], i16)
            nc.sync.dma_start(idx_sb[:], idx_t[:])
            dloc_sb = cpool.tile([P, TS // P], bf16)
            nc.sync.dma_start(dloc_sb[:], dloc_t[:])
            invdb_sb = cpool.tile([P, NPC], f32)
            nc.sync.dma_start(invdb_sb[:], invdb_t[:])
            xT_sb = cpool.tile([P, NPC], bf16)
            nc.sync.dma_start(xT_sb[:], xT_t[:])
            hT_sb = cpool.tile([P, NPC], bf16)
            W1s_sb = cpool.tile([D_IN, D_HID], bf16)
            nc.sync.dma_start(W1s_sb[:], W1s_t[:])
            W1n_sb = cpool.tile([D_IN, D_HID], bf16)
            nc.sync.dma_start(W1n_sb[:], W1n_t[:])
            b1_sb = cpool.tile([D_HID, 1], f32)
            nc.sync.dma_start(b1_sb[:], b1_t[:])
            W2s_sb = cpool.tile([D_HID, D_OUT], bf16)
            nc.sync.dma_start(W2s_sb[:], W2s_t[:])
            W2n_sb = cpool.tile([D_HID, D_OUT], bf16)
            nc.sync.dma_start(W2n_sb[:], W2n_t[:])
            b2_sb = cpool.tile([D_OUT, 1], f32)
            nc.sync.dma_start(b2_sb[:], b2_t[:])

            qrr = [0]

            def gcalls(ms, t, nchunks, region0, win):
                # one dma_gather per (tile, window); split only if > MAXG
                s0 = region0
                S = nchunks * P
                done = 0
                while done < S:
                    n = min(MAXG, S - done)
                    a = s0 + done
                    nc.gpsimd.dma_gather(
                        out_ap=ms[:, done // P:(done + n) // P, :],
                        in_ap=win,
                        idxs_ap=idx_sb[:, a // 16:(a + n) // 16],
                        num_idxs=n,
                        num_idxs_reg=n,
                        elem_size=D_IN,
                        queue_num=qrr[0] % 4,
                    )
                    qrr[0] += 1
                    done += n

            def layer(li, table_t):
                cc_done = set()
                winA = table_t[0:A_END, :]
                winB = table_t[B_OFF:NPAD, :]
                for t in range(NT):
                    ca, cb = CA[t], CB[t]
                    ct = ca + cb
                    msA = poolA.tile([P, CAMAX, P], bf16, name=f"msA{li}{t}",
                                     tag="msA")
                    msB = poolB.tile([P, CBMAX, P], bf16, name=f"msB{li}{t}",
                                     tag="msB")
                    gcalls(msA, t, ca, off[t], winA)
                    gcalls(msB, t, cb, off[t] + ca * P, winB)

                    oh = pool_oh.tile([P, CMAX, P], bf16, name=f"oh{li}{t}",
                                      tag="oh")
                    co = off[t] // P
                    nc.vector.tensor_tensor(
                        out=oh[:, :ct, :], in0=iota_sb[:, :ct, :],
                        in1=dloc_sb[:, co:co + ct].to_broadcast([P, ct, P]),
                        op=ALU.is_equal)

                    agg = psumA.tile([P, P], f32, name=f"agg{li}{t}", tag="agg")
                    for c in range(ct):
                        lhs = msA[:, c, :] if c < ca else msB[:, c - ca, :]
                        nc.tensor.matmul(
                            out=agg[:],
                            lhsT=lhs,
                            rhs=oh[:, c, :],
                            start=(c == 0),
                            stop=(c == ct - 1),
                        )
                    aggm = pool_aggm.tile([P, P], bf16, name=f"am{li}{t}",
                                          tag="aggm")
                    ncol = slice(t * P, (t + 1) * P)
                    nc.vector.tensor_tensor(out=aggm[:], in0=agg[:],
                                            in1=invdb_sb[:, ncol],
                                            op=ALU.mult)
                    if li == 0:
                        hps = psumH.tile([P, P], f32, name=f"h{t}", tag="hps")
                        nc.tensor.matmul(out=hps[:], lhsT=W1n_sb[:],
                                         rhs=aggm[:], start=True, stop=False)
                        nc.tensor.matmul(out=hps[:], lhsT=W1s_sb[:],
                                         rhs=xT_sb[:, ncol],
                                         start=False, stop=True)
                        nc.scalar.activation(hT_sb[:, ncol], hps[:], AF.Relu,
                                             bias=b1_sb[:, 0:1])
                        hnode = pool_small.tile([P, P], bf16, name=f"hn{t}",
                                                tag="hnode")
                        nc.sync.dma_start_transpose(hnode[:], hT_sb[:, ncol])
                        nc.sync.dma_start(
                            out=h_shard_t[t * P:(t + 1) * P, :], in_=hnode[:])
                        # chunk collective, issued a few tiles late so the
                        # in-order gpsimd queue (gather prefetch) doesn't
                        # stall on the chunk's store semaphores
                        for k in range(NCH):
                            if t == min(CH_T0[k + 1] - 1 + 6, NT - 1) and \
                                    k not in cc_done:
                                cc_done.add(k)
                                r0, r1 = CH_T0[k] * P, CH_T0[k + 1] * P
                                sz = r1 - r0
                                nc.gpsimd.collective_compute(
                                    "AllGather",
                                    mybir.AluOpType.bypass,
                                    replica_groups=[list(range(CORES))],
                                    ins=[h_shard_t[r0:r1, :]],
                                    outs=[h_table_t[CH_BASE[k]:
                                                    CH_BASE[k] + CORES * sz, :]],
                                )
                    else:
                        ops = psumH.tile([D_OUT, P], f32, name=f"o{t}",
                                         tag="hps")
                        nc.tensor.matmul(out=ops[:], lhsT=W2n_sb[:],
                                         rhs=aggm[:], start=True, stop=False)
                        nc.tensor.matmul(out=ops[:], lhsT=W2s_sb[:],
                                         rhs=hT_sb[:, ncol],
                                         start=False, stop=True)
                        osb = pool_small.tile([D_OUT, P], f32, name=f"os{t}",
                                              tag="osb")
                        nc.scalar.activation(osb[:], ops[:], AF.Identity,
                                             bias=b2_sb[:, 0:1])
                        nc.sync.dma_start(out=out_t[:, ncol], in_=osb[:])

            layer(0, x_bf_t)
            layer(1, h_table_t)

    nc.compile()
    return nc


_CACHE = {}


def kernel(x, src, dst, W1_self, W1_neigh, b1, W2_self, W2_neigh, b2,
           _want_perf=False):
    from concourse.bass_utils import run_bass_kernel_spmd

    per_core, meta = _prep(x, src, dst, W1_self, W1_neigh, b1,
                           W2_self, W2_neigh, b2)

    ck = (meta["CA"], meta["CB"])
    if ck not in _CACHE:
        _CACHE[ck] = _build(meta)
    nc = _CACHE[ck]

    res = run_bass_kernel_spmd(nc, per_core, core_ids=list(range(CORES)),
                               trace=_want_perf)

    node_of_g = meta["node_of_g"]
    outT = np.concatenate([r["outT"] for r in res.results], axis=1)  # [64, NPAD]
    out = np.empty((N, D_OUT), np.float32)
    valid = node_of_g >= 0
    out[node_of_g[valid]] = outT.T[valid]
    if _want_perf:
        return out, res
    return out


# revision 6
# speedup vs baseline: 1.0395x; 1.0395x over previous
"""GraphSAGE 2-layer GNN kernel for Trainium2 (8 NeuronCores, SPMD).

Strategy (dst-sharded graph parallel, v2):
  - Nodes are permuted (degree-balanced round-robin) into 392 tiles of 128
    nodes; each core owns 49 tiles.  Tables (x, h) live in DRAM in a
    "chunk-major" row space: 4 chunks of tiles (13/13/13/10 per core), each
    chunk holding all 8 cores' tiles contiguously, so the inter-layer
    AllGather runs as 4 pipelined chunk collectives that overlap layer-1
    compute.
  - int16 gather indices can't span all 50176 rows, so gathers use two
    windows aligned to chunk boundaries: A = rows [0, 26624) (chunks 0-1),
    B = rows [26624, 50176) (chunks 2-3).
  - Per (tile, window) one dma_gather call pulls that tile's source rows
    (bf16, 256 B/row) from the replicated table; variable per-tile chunk
    counts (max across cores, so all 8 cores share one program) keep
    padding low.  A larger SWDGE descriptor ring (dynamic_dma_scratch_size)
    allows ~1700-index calls.
  - Scatter-into-dst is one-hot matmul: per tile ONE DVE is_equal builds the
    combined [128, C_t, 128] one-hot (edge-slot x dst-slot, scaled later by
    1/deg), and C_t chunk matmuls accumulate aggT[d,:] in PSUM.
  - Layer output stays transposed (hT = W_self.T @ xT + W_neigh.T @ aggmeanT
    + b) in bf16; node-major bf16 copies for the collective come from a
    per-tile SBUF DMA transpose.

The final output is returned transposed per core ([64, 6272] fp32); the
host assembles/unpermutes to the full [50000, 64] result.
"""

import numpy as np
import ml_dtypes

N = 50000
E = 800000
D_IN = 128
D_HID = 128
D_OUT = 64
CORES = 8
P = 128

NT = 49                    # node tiles per core
NPC = NT * P               # padded nodes per core (6272)
NTILES = CORES * NT        # 392 total tiles
NPAD = CORES * NPC         # 50176 padded rows total

CH_T0 = [0, 13, 26, 39, 49]          # per-core tile chunk boundaries
NCH = 4
CH_BASE = [0, 13312, 26624, 39936]   # table row base of each chunk
A_END = 26624                        # window A rows [0, A_END)
B_OFF = 26624                        # window B rows [B_OFF, NPAD)

SCRATCH = 49152            # SWDGE descriptor carveout bytes/partition
MAXG = 1024                # max indices per dma_gather call

BF16 = ml_dtypes.bfloat16


def _prep(x, src, dst, W1_self, W1_neigh, b1, W2_self, W2_neigh, b2):
    """Host-side preprocessing: permutation, edge slotting, per-core arrays."""
    x = np.asarray(x, np.float32)
    src = np.asarray(src, np.int64)
    dst = np.asarray(dst, np.int64)

    deg = np.bincount(dst, minlength=N)
    invdeg = 1.0 / np.maximum(deg, 1).astype(np.float32)

    # Degree-balanced node -> (tile, slot): round-robin nodes in descending-
    # degree order over all 392 tiles. g = tile-major padded id.
    order = np.argsort(-deg, kind="stable")
    ranks = np.arange(N)
    g_of_node = np.empty(N, np.int64)
    g_of_node[order] = (ranks % NTILES) * P + ranks // NTILES
    node_of_g = np.full(NPAD, -1, np.int64)
    node_of_g[g_of_node] = np.arange(N)

    # chunk-major physical table row for every g
    g_all = np.arange(NPAD)
    core_g = g_all // NPC
    pos_g = (g_all % NPC) // P
    slot_g = g_all % P
    ch_of_pos = np.searchsorted(CH_T0, pos_g, side="right") - 1
    ch_sz = np.diff(CH_T0)
    row_of_g = (np.asarray(CH_BASE)[ch_of_pos]
                + core_g * ch_sz[ch_of_pos] * P
                + (pos_g - np.asarray(CH_T0)[ch_of_pos]) * P + slot_g)

    gsrc = g_of_node[src]
    gdst = g_of_node[dst]
    rsrc = row_of_g[gsrc]

    core_e = gdst // NPC
    t_e = (gdst % NPC) // P
    dloc_e = gdst % P
    half_e = (rsrc >= A_END).astype(np.int64)

    # per (core, tile, half) counts -> shared per-tile chunk counts
    key = ((core_e * NT + t_e) * 2 + half_e).astype(np.int64)
    counts = np.bincount(key, minlength=NTILES * 2).reshape(CORES, NT, 2)
    CA = np.maximum(1, np.ceil(counts[:, :, 0].max(axis=0) / P).astype(int))
    CB = np.maximum(1, np.ceil(counts[:, :, 1].max(axis=0) / P).astype(int))
    CT = CA + CB
    off = np.zeros(NT + 1, np.int64)
    off[1:] = np.cumsum(CT) * P           # slot offset of each tile region
    TS = int(off[-1])                      # slots per core
    CMAX = int(CT.max())

    # slot assignment: stable sort edges by key, place within group
    edge_order = np.argsort(key, kind="stable")
    key_s = key[edge_order]
    starts = np.zeros(NTILES * 2, np.int64)
    flat_counts = counts.reshape(-1)
    starts[1:] = np.cumsum(flat_counts)[:-1]
    within = np.arange(E) - starts[key_s]

    t_s = (key_s // 2) % NT
    half_s = key_s % 2
    slot_s = np.where(half_s == 0,
                      off[t_s] + within,
                      off[t_s] + CA[t_s] * P + within)
    core_s = key_s // (2 * NT)

    idx_arr = np.zeros((CORES, TS), np.int16)
    # pad slots: dloc sentinel 300 never matches any column 0..127 (exact
    # in bf16), so padded gather rows contribute nothing to the one-hot MM
    dloc_arr = np.full((CORES, TS), 300.0, np.float32)
    flat = core_s * TS + slot_s
    lidx_e = rsrc - half_e * B_OFF
    assert lidx_e.max() < 32768 and lidx_e.min() >= 0
    idx_arr.reshape(-1)[flat] = lidx_e[edge_order].astype(np.int16)
    dloc_arr.reshape(-1)[flat] = dloc_e[edge_order].astype(np.float32)

    # wrapped index layout: slot i -> [i % 16, i // 16], tiled to 128 rows
    idx_w = np.ascontiguousarray(
        np.tile(idx_arr.reshape(CORES, TS // 16, 16).transpose(0, 2, 1), (1, 8, 1))
    )
    # per-chunk scalar layout: slot i -> [i % 128, i // 128]
    dloc_w = np.ascontiguousarray(
        dloc_arr.reshape(CORES, TS // P, P).transpose(0, 2, 1)).astype(BF16)
    # per-node 1/deg, broadcast across partitions, per core shard [128, NPC]
    invdeg_pad = np.zeros(NPAD, np.float32)
    invdeg_pad[g_of_node] = invdeg
    invdb = [np.ascontiguousarray(
        np.tile(invdeg_pad[c * NPC:(c + 1) * NPC], (P, 1)))
        for c in range(CORES)]

    # feature tables / shards
    xpad = np.zeros((NPAD, D_IN), np.float32)
    xpad[g_of_node] = x
    x_tab = np.zeros((NPAD, D_IN), np.float32)
    x_tab[row_of_g] = xpad
    x_bf = x_tab.astype(BF16)
    xT_shards = [np.ascontiguousarray(xpad[c * NPC:(c + 1) * NPC].T).astype(BF16)
                 for c in range(CORES)]

    iota = np.tile(np.arange(P, dtype=np.float32), (P, CMAX)).astype(BF16)

    meta = dict(CA=tuple(int(v) for v in CA), CB=tuple(int(v) for v in CB),
                off=tuple(int(v) for v in off), TS=TS, CMAX=CMAX,
                node_of_g=node_of_g)

    common = {
        "x_bf": x_bf,
        "iota": np.ascontiguousarray(iota),
        "W1s": np.ascontiguousarray(np.asarray(W1_self, np.float32).astype(BF16)),
        "W1n": np.ascontiguousarray(np.asarray(W1_neigh, np.float32).astype(BF16)),
        "b1": np.ascontiguousarray(np.asarray(b1, np.float32).reshape(D_HID, 1)),
        "W2s": np.ascontiguousarray(np.asarray(W2_self, np.float32).astype(BF16)),
        "W2n": np.ascontiguousarray(np.asarray(W2_neigh, np.float32).astype(BF16)),
        "b2": np.ascontiguousarray(np.asarray(b2, np.float32).reshape(D_OUT, 1)),
    }
    per_core = []
    for c in range(CORES):
        m = dict(common)
        m["xT"] = xT_shards[c]
        m["idx_w"] = idx_w[c]
        m["dloc"] = dloc_w[c]
        m["invdb"] = invdb[c]
        per_core.append(m)
    return per_core, meta


def _build(meta):
    """Build the SPMD Bass program (same NEFF for all 8 cores)."""
    import concourse.bacc as bacc
    import concourse.mybir as mybir
    import concourse.tile as tile

    CA, CB, off = meta["CA"], meta["CB"], meta["off"]
    TS, CMAX = meta["TS"], meta["CMAX"]
    f32 = mybir.dt.float32
    bf16 = mybir.dt.bfloat16
    i16 = mybir.dt.int16
    AF = mybir.ActivationFunctionType
    ALU = mybir.AluOpType

    nc = bacc.Bacc(None, target_bir_lowering=False, debug=False,
                   num_devices=CORES, num_swdge_queues=4,
                   dynamic_dma_scratch_size=SCRATCH)

    # I/O
    x_bf_t = nc.dram_tensor("x_bf", [NPAD, D_IN], bf16, kind="ExternalInput")
    xT_t = nc.dram_tensor("xT", [P, NPC], bf16, kind="ExternalInput")
    idx_t = nc.dram_tensor("idx_w", [P, TS // 16], i16, kind="ExternalInput")
    dloc_t = nc.dram_tensor("dloc", [P, TS // P], bf16, kind="ExternalInput")
    invdb_t = nc.dram_tensor("invdb", [P, NPC], f32, kind="ExternalInput")
    iota_t = nc.dram_tensor("iota", [P, CMAX * P], bf16, kind="ExternalInput")
    W1s_t = nc.dram_tensor("W1s", [D_IN, D_HID], bf16, kind="ExternalInput")
    W1n_t = nc.dram_tensor("W1n", [D_IN, D_HID], bf16, kind="ExternalInput")
    b1_t = nc.dram_tensor("b1", [D_HID, 1], f32, kind="ExternalInput")
    W2s_t = nc.dram_tensor("W2s", [D_HID, D_OUT], bf16, kind="ExternalInput")
    W2n_t = nc.dram_tensor("W2n", [D_HID, D_OUT], bf16, kind="ExternalInput")
    b2_t = nc.dram_tensor("b2", [D_OUT, 1], f32, kind="ExternalInput")
    out_t = nc.dram_tensor("outT", [D_OUT, NPC], f32, kind="ExternalOutput")

    h_shard_t = nc.dram_tensor("h_shard", [NPC, D_HID], bf16)
    h_table_t = nc.dram_tensor("h_table", [NPAD, D_HID], bf16,
                               addr_space="Shared")

    CAMAX = max(CA)
    CBMAX = max(CB)

    with tile.TileContext(nc) as tc:
        with (
            tc.tile_pool(name="const", bufs=1) as cpool,
            tc.tile_pool(name="msgsA", bufs=9) as poolA,
            tc.tile_pool(name="msgsB", bufs=9) as poolB,
            tc.tile_pool(name="oh", bufs=4) as pool_oh,
            tc.tile_pool(name="aggm", bufs=4) as pool_aggm,
            tc.tile_pool(name="small", bufs=3) as pool_small,
            tc.tile_pool(name="psA", bufs=3, space="PSUM") as psumA,
            tc.tile_pool(name="psH", bufs=2, space="PSUM") as psumH,
        ):
            # ---- persistent SBUF state -------------------------------------
            iota_sb = cpool.tile([P, CMAX, P], bf16)
            nc.sync.dma_start(iota_sb[:, :, :], iota_t[:].rearrange(
                "p (c n) -> p c n", n=P))
            idx_sb = cpool.tile([P, TS // 16], i16)
            nc.sync.dma_start(idx_sb[:], idx_t[:])
            dloc_sb = cpool.tile([P, TS // P], bf16)
            nc.sync.dma_start(dloc_sb[:], dloc_t[:])
            invdb_sb = cpool.tile([P, NPC], f32)
            nc.sync.dma_start(invdb_sb[:], invdb_t[:])
            xT_sb = cpool.tile([P, NPC], bf16)
            nc.sync.dma_start(xT_sb[:], xT_t[:])
            hT_sb = cpool.tile([P, NPC], bf16)
            W1s_sb = cpool.tile([D_IN, D_HID], bf16)
            nc.sync.dma_start(W1s_sb[:], W1s_t[:])
            W1n_sb = cpool.tile([D_IN, D_HID], bf16)
            nc.sync.dma_start(W1n_sb[:], W1n_t[:])
            b1_sb = cpool.tile([D_HID, 1], f32)
            nc.sync.dma_start(b1_sb[:], b1_t[:])
            W2s_sb = cpool.tile([D_HID, D_OUT], bf16)
            nc.sync.dma_start(W2s_sb[:], W2s_t[:])
            W2n_sb = cpool.tile([D_HID, D_OUT], bf16)
            nc.sync.dma_start(W2n_sb[:], W2n_t[:])
            b2_sb = cpool.tile([D_OUT, 1], f32)
            nc.sync.dma_start(b2_sb[:], b2_t[:])

            qrr = [0]

            def gcalls(ms, t, nchunks, region0, win):
                # one dma_gather per (tile, window); split only if > MAXG
                s0 = region0
                S = nchunks * P
                done = 0
                while done < S:
                    n = min(MAXG, S - done)
                    a = s0 + done
                    nc.gpsimd.dma_gather(
                        out_ap=ms[:, done // P:(done + n) // P, :],
                        in_ap=win,
                        idxs_ap=idx_sb[:, a // 16:(a + n) // 16],
                        num_idxs=n,
                        num_idxs_reg=n,
                        elem_size=D_IN,
                        queue_num=qrr[0] % 4,
                    )
                    qrr[0] += 1
                    done += n

            def layer(li, table_t):
                winA = table_t[0:A_END, :]
                winB = table_t[B_OFF:NPAD, :]
                for t in range(NT):
                    ca, cb = CA[t], CB[t]
                    ct = ca + cb
                    msA = poolA.tile([P, CAMAX, P], bf16, name=f"msA{li}{t}",
                                     tag="msA")
                    msB = poolB.tile([P, CBMAX, P], bf16, name=f"msB{li}{t}",
                                     tag="msB")
                    gcalls(msA, t, ca, off[t], winA)
                    gcalls(msB, t, cb, off[t] + ca * P, winB)

                    oh = pool_oh.tile([P, CMAX, P], bf16, name=f"oh{li}{t}",
                                      tag="oh")
                    co = off[t] // P
                    nc.vector.tensor_tensor(
                        out=oh[:, :ct, :], in0=iota_sb[:, :ct, :],
                        in1=dloc_sb[:, co:co + ct].to_broadcast([P, ct, P]),
                        op=ALU.is_equal)

                    agg = psumA.tile([P, P], f32, name=f"agg{li}{t}", tag="agg")
                    for c in range(ct):
                        lhs = msA[:, c, :] if c < ca else msB[:, c - ca, :]
                        nc.tensor.matmul(
                            out=agg[:],
                            lhsT=lhs,
                            rhs=oh[:, c, :],
                            start=(c == 0),
                            stop=(c == ct - 1),
                        )
                    aggm = pool_aggm.tile([P, P], bf16, name=f"am{li}{t}",
                                          tag="aggm")
                    ncol = slice(t * P, (t + 1) * P)
                    nc.vector.tensor_tensor(out=aggm[:], in0=agg[:],
                                            in1=invdb_sb[:, ncol],
                                            op=ALU.mult)
                    if li == 0:
                        hps = psumH.tile([P, P], f32, name=f"h{t}", tag="hps")
                        nc.tensor.matmul(out=hps[:], lhsT=W1n_sb[:],
                                         rhs=aggm[:], start=True, stop=False)
                        nc.tensor.matmul(out=hps[:], lhsT=W1s_sb[:],
                                         rhs=xT_sb[:, ncol],
                                         start=False, stop=True)
                        nc.scalar.activation(hT_sb[:, ncol], hps[:], AF.Relu,
                                             bias=b1_sb[:, 0:1])
                        hnode = pool_small.tile([P, P], bf16, name=f"hn{t}",
                                                tag="hnode")
                        nc.sync.dma_start_transpose(hnode[:], hT_sb[:, ncol])
                        nc.sync.dma_start(
                            out=h_shard_t[t * P:(t + 1) * P, :], in_=hnode[:])
                    else:
                        ops = psumH.tile([D_OUT, P], f32, name=f"o{t}",
                                         tag="hps")
                        nc.tensor.matmul(out=ops[:], lhsT=W2n_sb[:],
                                         rhs=aggm[:], start=True, stop=False)
                        nc.tensor.matmul(out=ops[:], lhsT=W2s_sb[:],
                                         rhs=hT_sb[:, ncol],
                                         start=False, stop=True)
                        osb = pool_small.tile([D_OUT, P], f32, name=f"os{t}",
                                              tag="osb")
                        nc.scalar.activation(osb[:], ops[:], AF.Identity,
                                             bias=b2_sb[:, 0:1])
                        nc.sync.dma_start(out=out_t[:, ncol], in_=osb[:])

            layer(0, x_bf_t)
            for k in range(NCH):
                r0, r1 = CH_T0[k] * P, CH_T0[k + 1] * P
                sz = r1 - r0
                nc.gpsimd.collective_compute(
                    "AllGather",
                    mybir.AluOpType.bypass,
                    replica_groups=[list(range(CORES))],
                    ins=[h_shard_t[r0:r1, :]],
                    outs=[h_table_t[CH_BASE[k]:CH_BASE[k] + CORES * sz, :]],
                )
            layer(1, h_table_t)

    nc.compile()
    return nc


_CACHE = {}


def kernel(x, src, dst, W1_self, W1_neigh, b1, W2_self, W2_neigh, b2,
           _want_perf=False):
    from concourse.bass_utils import run_bass_kernel_spmd

    per_core, meta = _prep(x, src, dst, W1_self, W1_neigh, b1,
                           W2_self, W2_neigh, b2)

    ck = (meta["CA"], meta["CB"])
    if ck not in _CACHE:
        _CACHE[ck] = _build(meta)
    nc = _CACHE[ck]

    res = run_bass_kernel_spmd(nc, per_core, core_ids=list(range(CORES)),
                               trace=_want_perf)

    node_of_g = meta["node_of_g"]
    outT = np.concatenate([r["outT"] for r in res.results], axis=1)  # [64, NPAD]
    out = np.empty((N, D_OUT), np.float32)
    valid = node_of_g >= 0
    out[node_of_g[valid]] = outT.T[valid]
    if _want_perf:
        return out, res
    return out
